# revision 36
# baseline (speedup 1.0000x reference)
"""Trainium2 Bass kernel for nn_BlocksCore (RIMs BlocksCore forward).

Data-parallel over batch across 8 NeuronCores (512 samples/core).
Compute layout: feature-major [feat, batch] for matmuls (weights stationary),
sample-major [batch, feat] for the LSTM cell / top-k gating, where per-sample
scalars (attention mixing weight, block mask) map to [P,1] tensor-scalar ops.

Numerical strategy: the input-attention score path (q, k, q.dk reduction) runs
in full fp32 because the top-4/bottom-4 block ranking has a min margin of
~1.8e-4 over the 4096 samples (fp32r matmuls, ~1.6e-4 relative error, would
flip masks). The dominant LSTM-gate matmuls run in fp32r (full PE rate at
N=512); fp32r operands must be produced as fp32r (walrus dataflow check), so
the hx stationary operand gets a one-time rounded copy.

Input attention is algebraically reduced: with 2 key blocks, softmax weights
are (1-a, a) with a = sigmoid(e1 - e0), so inp_use = v0 + a*(v1 - v0) and the
block-diagonal LSTM input projection becomes
    gates_ih_j = v0 @ WihT_j + (a_j * dv) @ WihT_j,
where the per-sample scale a_j is applied to dv's stationary-operand columns
(samples) via a PE row-select broadcast matmul. Everything accumulates into a
single PSUM tile per (block, sample-block).
"""

import threading
from contextlib import ExitStack

import numpy as np

import concourse.bass as bass
import concourse.mybir as mybir
import concourse.tile as tile
from concourse.bass_utils import run_bass_kernel_spmd
from concourse.masks import make_identity
from concourse.vector_clock import ScopedClock

F32 = mybir.dt.float32
F32R = mybir.dt.float32r
ALU = mybir.AluOpType
ACTF = mybir.ActivationFunctionType
AX = mybir.AxisListType

B = 4096
NCORES = 8
BPC = B // NCORES            # 512 samples per core
NSB = BPC // 128             # 4 sample blocks of 128
NHID = 1024
BS_IN = 512
BS_OUT = 128
ATT_OUT = 512
DK_IN = 64
INV_SQRT_DK_IN = 1.0 / 8.0
INV_SQRT_DK_C = float(1.0 / np.sqrt(32.0))


# ---------------------------------------------------------------------------
# Workarounds: this walrus build accepts at most ONE semaphore wait per
# instruction. (1) split the Tile tail-drain's waits across sequential SP
# drains; (2) after scheduling, hoist extra waits onto same-engine NOPs.
# ---------------------------------------------------------------------------
def _patched_drain_and_barrier(self, tick_clock, wait_clock):
    nc = self.nc
    drain_inst = nc.sync.drain()
    wait_clock.add_sem_waits(
        drain_inst.ins, ScopedClock({None: tick_clock.global_clock})
    )
    si = drain_inst.ins.sync_info
    if si is not None and si.on_wait is not None and len(si.on_wait) > 1:
        waits = list(si.on_wait)
        drain_inst.ins.sync_info = mybir.SyncInfo(
            on_wait=waits[:1], on_update=list(si.on_update or [])
        )
        for w in waits[1:]:
            d2 = nc.sync.drain()
            d2.ins.sync_info = mybir.SyncInfo(on_wait=[w], on_update=[])

    nc.all_engine_barrier()
    assert self.sems is not None
    popped = nc._tile_sem_poison_stack.pop()
    assert popped is self._sem_poison
    nc.clear_and_free_semaphores(list(self.sems.allocated().values()))
    nc.all_engine_barrier()


_ORIG_LOWER = tile.TileContext._lower_ordered_insts
_NOPID = [0]


def _split_multiwait_lower(self, ordered):
    for bb in list(ordered.keys()):
        out = []
        for inst in ordered[bb]:
            si = getattr(inst, "sync_info", None)
            if si is not None and si.on_wait is not None and len(si.on_wait) > 1:
                waits = list(si.on_wait)
                for w in waits[:-1]:
                    _NOPID[0] += 1
                    out.append(mybir.InstNoOp(
                        name=f"{inst.name}_mw{_NOPID[0]}",
                        sync_info=mybir.SyncInfo(on_wait=[w], on_update=[]),
                        bass_nofuse=True,
                        engine=inst.engine,
                    ))
                inst.sync_info = mybir.SyncInfo(
                    on_wait=[waits[-1]], on_update=list(si.on_update or []))
            out.append(inst)
        ordered[bb] = out
    return _ORIG_LOWER(self, ordered)


def _apply_tile_patch():
    tile.TileContext._drain_and_barrier = _patched_drain_and_barrier
    tile.TileContext._lower_ordered_insts = _split_multiwait_lower


def _r(ap):
    return ap.bitcast(F32R)


# ---------------------------------------------------------------------------
# Device kernel body
# ---------------------------------------------------------------------------
def build_kernel(ctx, tc, io, use_bias):
    nc = tc.nc

    consts = ctx.enter_context(tc.tile_pool(name="consts", bufs=1))
    acts = ctx.enter_context(tc.tile_pool(name="acts", bufs=1))
    wstream = ctx.enter_context(tc.tile_pool(name="wstream", bufs=2))
    tbig = ctx.enter_context(tc.tile_pool(name="tbig", bufs=1))
    tsmall = ctx.enter_context(tc.tile_pool(name="tsmall", bufs=2))
    cxp = ctx.enter_context(tc.tile_pool(name="cxp", bufs=3))
    dvsp = ctx.enter_context(tc.tile_pool(name="dvsp", bufs=4))
    qkvp = ctx.enter_context(tc.tile_pool(name="qkvp", bufs=1))
    outp = ctx.enter_context(tc.tile_pool(name="outp", bufs=3))
    mout = ctx.enter_context(tc.tile_pool(name="mout", bufs=1))
    ps_gate = ctx.enter_context(
        tc.tile_pool(name="ps_gate", bufs=2, space="PSUM"))
    ps_big = ctx.enter_context(tc.tile_pool(name="ps_big", bufs=2, space="PSUM"))
    ps_small = ctx.enter_context(
        tc.tile_pool(name="ps_small", bufs=3, space="PSUM"))
    ps_tr = ctx.enter_context(tc.tile_pool(name="ps_tr", bufs=1, space="PSUM"))

    # ---- constants -------------------------------------------------------
    ident = consts.tile([128, 128], F32, tag="ident")
    make_identity(nc, ident)

    ones_col = consts.tile([1, 128], F32, tag="ones_col")
    nc.vector.memset(ones_col, 1.0)

    # host-provided row-select matrices (bigE[:, r, :] is [16, 128] with row
    # r all-ones): a matmul with it as lhsT broadcasts row r of a [16, N] rhs
    # across 128 output partitions.
    bigE = consts.tile([16, 16, 128], F32R, tag="bigE")
    nc.sync.dma_start(out=bigE, in_=io["e16"].bitcast(F32R))

    # lower-triangular [j, i] -> 1.0 iff i < j  (tie-break mask for top-k)
    iot_i = consts.tile([128, 8, 8], F32, tag="iot_i")
    iot_j = consts.tile([128, 8, 8], F32, tag="iot_j")
    nc.gpsimd.iota(iot_i, pattern=[[0, 8], [1, 8]], base=0,
                   channel_multiplier=0, allow_small_or_imprecise_dtypes=True)
    nc.gpsimd.iota(iot_j, pattern=[[1, 8], [0, 8]], base=0,
                   channel_multiplier=0, allow_small_or_imprecise_dtypes=True)
    lt8 = consts.tile([128, 8, 8], F32, tag="lt8")
    nc.vector.tensor_tensor(lt8, iot_i, iot_j, ALU.is_lt)

    # ---- load activations ------------------------------------------------
    xT = []
    for c in range(8):
        t = acts.tile([128, BPC], F32, tag=f"xT{c}")
        nc.sync.dma_start(out=t, in_=io["xT"][c * 128:(c + 1) * 128, :])
        xT.append(t)
    hxT = []
    for c in range(8):
        t = acts.tile([128, BPC], F32, tag=f"hxT{c}")
        nc.sync.dma_start(out=t, in_=io["hxT"][c * 128:(c + 1) * 128, :])
        hxT.append(t)

    # ---- load weights ----------------------------------------------------
    wq = consts.tile([128, DK_IN], F32, tag="wq")
    nc.sync.dma_start(out=wq, in_=io["wq"][:])
    wk = consts.tile([128, 4, DK_IN], F32, tag="wk")
    nc.sync.dma_start(out=wk, in_=io["wk"].rearrange("(c p) d -> p c d", p=128))
    wv = consts.tile([128, 4, ATT_OUT], F32, tag="wv")
    nc.sync.dma_start(out=wv, in_=io["wv"].rearrange("(c p) d -> p c d", p=128))
    bvt = consts.tile([128, 4], F32, tag="bvt")
    nc.sync.dma_start(out=bvt, in_=io["bv"].rearrange("(c p) -> p c", p=128))

    wc = {}
    for nm in ("wqc", "wkc", "wvc", "wfc", "wgc"):
        t = consts.tile([128, 128], F32, tag=nm)
        nc.sync.dma_start(out=t, in_=io[nm][:])
        wc[nm] = t

    # ---- dx = x1 - x0 ----------------------------------------------------
    dxT = []
    for c in range(4):
        t = acts.tile([128, BPC], F32, tag=f"dxT{c}")
        nc.vector.tensor_tensor(t, xT[4 + c], xT[c], ALU.subtract)
        dxT.append(t)

    # ---- scores, iatt1, mask (sample-major, full fp32) ------------------
    # mi16[sb][:, 0:8] = block mask, mi16[sb][:, 8:16] = iatt1
    mi16 = []
    for sb in range(NSB):
        sbs = slice(sb * 128, (sb + 1) * 128)
        ps_q = ps_big.tile([128, 512], F32, tag="big", name=f"psq{sb}")
        for j in range(8):
            nc.tensor.matmul(ps_q[:, j * 64:(j + 1) * 64], hxT[j][:, sbs], wq,
                             start=True, stop=(not use_bias))
            if use_bias:
                brow = tsmall.tile([1, 64], F32, tag="bias_row",
                                   name=f"bqi{j}_{sb}")
                nc.sync.dma_start(out=brow, in_=io["bqi"][None, :])
                nc.tensor.matmul(ps_q[:, j * 64:(j + 1) * 64], ones_col, brow,
                                 start=False, stop=True)
        ps_k = ps_small.tile([128, 64], F32, tag="cmm", name=f"psk{sb}")
        for c in range(4):
            nc.tensor.matmul(ps_k, dxT[c][:, sbs], wk[:, c, :],
                             start=(c == 0), stop=(c == 3))
        # ndk = k1 - k0, so s' = q . ndk / 8 = e1 - e0 (negated score)
        dk = tsmall.tile([128, 64], F32, tag="dk", name=f"dk{sb}")
        nc.scalar.copy(dk, ps_k)

        s_sb = tsmall.tile([128, 8], F32, tag="s_sb", name=f"s{sb}")
        junk = tsmall.tile([128, 64], F32, tag="junk", name=f"junk{sb}")
        for j in range(8):
            nc.vector.scalar_tensor_tensor(
                junk, ps_q[:, j * 64:(j + 1) * 64], INV_SQRT_DK_IN, dk,
                ALU.mult, ALU.mult, accum_out=s_sb[:, j:j + 1])

        mi = acts.tile([128, 16], F32, tag=f"mi16_{sb}")
        # s' = e1 - e0, so iatt1 = sigmoid(s')
        nc.scalar.activation(mi[:, 8:16], s_sb, ACTF.Sigmoid)

        # s' = -s: bottom-4 of s are the top-4 of s'. rank'_j =
        # #{i: s'_i > s'_j} + #{i<j: s'_i == s'_j}; keep rank' >= 4
        pm = tsmall.tile([128, 8, 8], F32, tag="pm", name=f"pm{sb}")
        pe = tsmall.tile([128, 8, 8], F32, tag="pe", name=f"pe{sb}")
        s_bi = s_sb[:, None, :].to_broadcast([128, 8, 8])   # s_i along inner
        s_bj = s_sb[:, :, None].to_broadcast([128, 8, 8])   # s_j along outer
        nc.vector.tensor_tensor(pm, s_bi, s_bj, ALU.is_gt)
        nc.vector.tensor_tensor(pe, s_bi, s_bj, ALU.is_equal)
        nc.vector.tensor_tensor(pe, pe, lt8, ALU.mult)
        nc.vector.tensor_tensor(pm, pm, pe, ALU.add)
        cnt = tsmall.tile([128, 8], F32, tag="cnt", name=f"cnt{sb}")
        nc.vector.reduce_sum(cnt, pm, axis=AX.X)
        nc.vector.tensor_scalar(mi[:, 0:8], cnt, 4.0, None, ALU.is_ge)
        mi16.append(mi)

        mbc = mout.tile([128, NHID], F32, tag="mbc", name=f"mbc{sb}")
        nc.vector.tensor_copy(mbc,
                              mi[:, 0:8, None].to_broadcast([128, 8, 128]))
        nc.sync.dma_start(out=io["mask_out"][sbs, :], in_=mbc)
        nc.sync.dma_start(out=io["bm_out"][sbs, :], in_=mi[:, 0:8])

    # hx stationary operand for the fp32r gate matmuls must be produced as
    # fp32r: one-time rounded copies (score path above used full-fp32 hxT)
    hxTr = []
    for c in range(8):
        t = acts.tile([128, BPC], F32R, tag=f"hxTr{c}")
        nc.vector.tensor_copy(t, hxT[c])
        hxTr.append(t)

    # miT: feature-major [16, BPC]; row j = mask_j, row 8+j = iatt1_j
    miT = acts.tile([16, BPC], F32R, tag="miT")
    for sb in range(NSB):
        pst = ps_tr.tile([128, 128], F32, tag="tr", name=f"mtr{sb}")
        nc.tensor.transpose(pst[0:16, :], mi16[sb], ident)
        nc.scalar.copy(miT[:, sb * 128:(sb + 1) * 128], pst[0:16, :])

    # ---- v0T, dvT (full fp32 matmuls; x/dx stay fp32 regions) -----------
    v0T, dvT = [], []
    for m in range(4):
        ps = ps_big.tile([128, BPC], F32, tag="big", name=f"psv0_{m}")
        for c in range(4):
            nc.tensor.matmul(ps, wv[:, c, m * 128:(m + 1) * 128],
                             xT[c], start=(c == 0), stop=(c == 3))
        t = acts.tile([128, BPC], F32R, tag=f"v0T{m}")
        nc.scalar.activation(t, ps, ACTF.Identity, bias=bvt[:, m:m + 1],
                             scale=1.0)
        v0T.append(t)
    for m in range(4):
        ps = ps_big.tile([128, BPC], F32, tag="big", name=f"psdv_{m}")
        for c in range(4):
            nc.tensor.matmul(ps, wv[:, c, m * 128:(m + 1) * 128],
                             dxT[c], start=(c == 0), stop=(c == 3))
        t = acts.tile([128, BPC], F32, tag=f"dvT{m}")
        nc.scalar.copy(t, ps)
        dvT.append(t)

    # ---- gates, LSTM cell, cx blend (feature-major) ---------------------
    # hbT reuses the xT slots (xT is dead after the v matmuls; both fp32)
    hbT = []
    for j in range(8):
        t = acts.tile([128, BPC], F32, tag=f"xT{j}")
        hbT.append(t)

    GATE_ACT = [ACTF.Sigmoid, ACTF.Sigmoid, ACTF.Tanh, ACTF.Sigmoid]
    for j in range(8):
        wih = wstream.tile([128, 4, 4, 128], F32R, tag="wih", name=f"wih{j}")
        nc.sync.dma_start(
            out=wih,
            in_=io["wihT"][j].rearrange("(c p) (gc go) -> p c gc go",
                                        p=128, go=128).bitcast(F32R))
        whh = wstream.tile([128, 4, 128], F32R, tag="whh", name=f"whh{j}")
        nc.sync.dma_start(
            out=whh,
            in_=io["whhT"][j].rearrange("p (gc go) -> p gc go",
                                        go=128).bitcast(F32R))
        cxTj = cxp.tile([128, BPC], F32, tag="cxT", name=f"cxT{j}")
        nc.sync.dma_start(out=cxTj, in_=io["cxT"][j * 128:(j + 1) * 128, :])

        # iatt1_j broadcast feature-major: bcA = row (8+j) of miT
        bcA = ps_big.tile([128, BPC], F32, tag="big", name=f"bcA{j}")
        nc.tensor.matmul(bcA, bigE[:, 8 + j, :], miT, start=True, stop=True)
        # dvs_c = iatt1_j * dvT_c  (scales the moving-operand columns)
        dvs = []
        for c in range(4):
            t = dvsp.tile([128, BPC], F32R, tag="dvs", name=f"dvs{j}_{c}")
            nc.vector.tensor_tensor(t, dvT[c], bcA, ALU.mult)
            dvs.append(t)

        # gates feature-major: one [128, BPC] tile per gate (i, f, g, o);
        # moving operand = activations (F32R), stationary = weight chunks
        gact = []
        for gc in range(4):
            psA = ps_gate.tile([128, BPC], F32, tag="psA", name=f"psA{j}_{gc}")
            for c in range(4):
                nc.tensor.matmul(psA, wih[:, c, gc, :], v0T[c],
                                 start=(c == 0), stop=False)
            for c in range(4):
                nc.tensor.matmul(psA, wih[:, c, gc, :], dvs[c],
                                 start=False, stop=False)
            nc.tensor.matmul(psA, whh[:, gc, :], hxTr[j],
                             start=False, stop=(not use_bias))
            if use_bias:
                bg_row = tsmall.tile([1, BPC], F32R, tag="bg_row",
                                     name=f"bg{j}_{gc}")
                nc.sync.dma_start(
                    out=bg_row,
                    in_=io["biasgT"][j, gc, :, None]
                    .to_broadcast([1, BPC]).bitcast(F32R))
                nc.tensor.matmul(psA, _r(ones_col), bg_row,
                                 start=False, stop=True)
            g = tsmall.tile([128, BPC], F32, tag=f"gact{gc}",
                            name=f"g{j}_{gc}")
            nc.scalar.activation(g, psA, GATE_ACT[gc])
            gact.append(g)

        sigi, sigf, tng, sgo = gact
        t1 = tbig.tile([128, BPC], F32, tag="t1", name=f"t1_{j}")
        nc.vector.tensor_tensor(t1, sigf, cxTj, ALU.mult)
        t2 = tbig.tile([128, BPC], F32, tag="t2", name=f"t2_{j}")
        nc.vector.tensor_tensor(t2, sigi, tng, ALU.mult)
        cxn = tbig.tile([128, BPC], F32, tag="cxn", name=f"cxn{j}")
        nc.vector.tensor_tensor(cxn, t1, t2, ALU.add)
        tnc = tbig.tile([128, BPC], F32, tag="tnc", name=f"tnc{j}")
        nc.scalar.activation(tnc, cxn, ACTF.Tanh)
        nc.vector.tensor_tensor(hbT[j], sgo, tnc, ALU.mult)

        # cx blend: cx_out = cx + mask_j * (cx_new - cx), feature-major
        mexpj = ps_small.tile([128, BPC], F32, tag="cmm", name=f"mexG{j}")
        nc.tensor.matmul(mexpj, bigE[:, j, :], miT, start=True, stop=True)
        nc.gpsimd.tensor_tensor(cxn, cxn, cxTj, ALU.subtract)
        dcm = tbig.tile([128, BPC], F32, tag="dcm", name=f"dcm{j}")
        nc.vector.tensor_tensor(dcm, cxn, mexpj, ALU.mult)
        cxo = outp.tile([128, BPC], F32, tag="cxo", name=f"cxo{j}")
        nc.gpsimd.tensor_tensor(cxo, dcm, cxTj, ALU.add)
        nc.sync.dma_start(out=io["cx_outT"][j * 128:(j + 1) * 128, :],
                          in_=cxo)

    # ---- communication attention + output fc + hx blend (per sb) --------
    for sb in range(NSB):
        sbs = slice(sb * 128, (sb + 1) * 128)
        qc, kc, vc = [], [], []
        for j in range(8):
            psq = ps_small.tile([128, 128], F32, tag="cmm",
                                name=f"pq{j}_{sb}")
            psk = ps_small.tile([128, 128], F32, tag="cmm",
                                name=f"pk{j}_{sb}")
            psv = ps_small.tile([128, 128], F32, tag="cmm",
                                name=f"pv{j}_{sb}")
            lhsT = hbT[j][:, sbs]
            nc.tensor.matmul(psq, lhsT, wc["wqc"],
                             start=True, stop=(not use_bias))
            nc.tensor.matmul(psk, lhsT, wc["wkc"],
                             start=True, stop=(not use_bias))
            nc.tensor.matmul(psv, lhsT, wc["wvc"],
                             start=True, stop=(not use_bias))
            if use_bias:
                for ps, bn in ((psq, "bqc"), (psk, "bkc"), (psv, "bvc")):
                    brow = tsmall.tile([1, 128], F32, tag="brow",
                                       name=f"b{bn}{j}_{sb}")
                    nc.sync.dma_start(out=brow, in_=io[bn][None, :])
                    nc.tensor.matmul(ps, ones_col, brow, start=False,
                                     stop=True)
            if j == 0:
                qcall = qkvp.tile([128, 8, 4, 32], F32, tag="qcall",
                                  name=f"qcall{sb}")
            tk = qkvp.tile([128, 4, 32], F32, tag=f"kc{j}")
            tv = qkvp.tile([128, 4, 32], F32, tag=f"vc{j}")
            nc.scalar.copy(qcall[:, j], psq.rearrange("p (h d) -> p h d", d=32))
            nc.scalar.copy(tk, psk.rearrange("p (h d) -> p h d", d=32))
            nc.scalar.copy(tv, psv.rearrange("p (h d) -> p h d", d=32))
            kc.append(tk)
            vc.append(tv)

        # scores S[b, h, qi, ki]: batched over qi per ki; muls split
        # across DVE and GPSIMD, segmented reduces on DVE
        S3 = tbig.tile([128, 4, 64], F32, tag="S3", name=f"S3_{sb}")
        S3r = S3.rearrange("p h (q k) -> p q h k", k=8)
        for ki in range(8):
            prodq = tbig.tile([128, 8, 4, 32], F32, tag=f"prodq{ki % 2}",
                              name=f"prod{sb}_{ki}")
            k_bc = kc[ki][:, None, :, :].to_broadcast([128, 8, 4, 32])
            eng = nc.vector if ki % 2 == 0 else nc.gpsimd
            eng.tensor_tensor(prodq, qcall, k_bc, ALU.mult)
            nc.vector.reduce_sum(S3r[:, :, :, ki], prodq, axis=AX.X)
        # softmax over ki (exp and normalize in place)
        A = S3.rearrange("p h (q k) -> p h q k", k=8)
        nc.scalar.activation(A, A, ACTF.Exp, scale=INV_SQRT_DK_C)
        den = tsmall.tile([128, 4, 8], F32, tag="den", name=f"den{sb}")
        nc.vector.reduce_sum(den, A, axis=AX.X)
        rec = tsmall.tile([128, 4, 8], F32, tag="rec", name=f"rec{sb}")
        nc.vector.reciprocal(rec, den)
        nc.vector.tensor_tensor(
            A, A, rec[:, :, :, None].to_broadcast([128, 4, 8, 8]), ALU.mult)

        # AV: o[b, qi, h, d] = sum_ki A[b,h,qi,ki] * vc[b,ki,(h,d)]
        o_a = tbig.tile([128, 8, 4, 32], F32, tag="o_a", name=f"oa{sb}")
        o_b = tbig.tile([128, 8, 4, 32], F32, tag="o_b", name=f"ob{sb}")
        cur = o_a
        for ki in range(8):
            prod2 = tbig.tile([128, 8, 4, 32], F32, tag=f"prodq{ki % 2}",
                              name=f"p2_{sb}_{ki}")
            a_sl = (A[:, :, :, ki]                   # [128, h, qi]
                    .rearrange("p h q -> p q h")     # [128, qi, h]
                    [:, :, :, None].to_broadcast([128, 8, 4, 32]))
            v_bc = vc[ki][:, None, :, :].to_broadcast([128, 8, 4, 32])
            eng = nc.vector if ki % 2 == 0 else nc.gpsimd
            if ki == 0:
                eng.tensor_tensor(cur, v_bc, a_sl, ALU.mult)
            else:
                eng.tensor_tensor(prod2, v_bc, a_sl, ALU.mult)
                nxt = o_b if cur is o_a else o_a
                nc.vector.tensor_tensor(nxt, cur, prod2, ALU.add)
                cur = nxt

        # per block: transpose o, output fc, gated tanh, hx blend
        for j in range(8):
            pst = ps_tr.tile([128, 128], F32, tag="tr", name=f"otr{j}_{sb}")
            nc.tensor.transpose(pst, cur[:, j], ident)
            otmp = tsmall.tile([128, 128], F32, tag="otmp",
                               name=f"ot{j}_{sb}")
            nc.scalar.copy(otmp, pst)

            psf = ps_small.tile([128, 128], F32, tag="cmm", name=f"psf{j}_{sb}")
            psg = ps_small.tile([128, 128], F32, tag="cmm", name=f"psg{j}_{sb}")
            nc.tensor.matmul(psf, wc["wfc"], otmp, start=True, stop=True)
            nc.tensor.matmul(psg, wc["wgc"], otmp, start=True, stop=True)
            tf = tsmall.tile([128, 128], F32, tag="tf", name=f"tf{j}_{sb}")
            sg = tsmall.tile([128, 128], F32, tag="sg", name=f"sg{j}_{sb}")
            if use_bias:
                bfcol = consts.tile([128, 1], F32, tag="bfcol")
                bgcol = consts.tile([128, 1], F32, tag="bgcol")
                if j == 0 and sb == 0:
                    nc.sync.dma_start(out=bfcol, in_=io["bfc"][:, None])
                    nc.sync.dma_start(out=bgcol, in_=io["bgc"][:, None])
                nc.scalar.activation(tf, psf, ACTF.Tanh, bias=bfcol, scale=1.0)
                nc.scalar.activation(sg, psg, ACTF.Sigmoid, bias=bgcol,
                                     scale=1.0)
            else:
                nc.scalar.activation(tf, psf, ACTF.Tanh)
                nc.scalar.activation(sg, psg, ACTF.Sigmoid)
            # comm = sigmoid(og) * tanh(of), in place on sg
            nc.vector.tensor_tensor(sg, sg, tf, ALU.mult)

            # hx_new = hb + comm; hx_out = hx + mask*(hx_new - hx)
            mexp = ps_small.tile([128, 128], F32, tag="cmm",
                                name=f"mexp{j}_{sb}")
            nc.tensor.matmul(mexp, bigE[:, j, :], miT[:, sbs],
                             start=True, stop=True)
            hb = hbT[j][:, sbs]
            nc.vector.tensor_tensor(hb, hb, sg, ALU.add)
            nc.vector.tensor_tensor(hb, hb, hxT[j][:, sbs], ALU.subtract)
            nc.vector.tensor_tensor(hb, hb, mexp, ALU.mult)
            hout = outp.tile([128, 128], F32, tag="hout", name=f"ho{j}_{sb}")
            nc.vector.tensor_tensor(hout, hb, hxT[j][:, sbs], ALU.add)
            nc.sync.dma_start(out=io["hx_outT"][j * 128:(j + 1) * 128, sbs],
                              in_=hout)


# ---------------------------------------------------------------------------
# Host side
# ---------------------------------------------------------------------------
_BUILD_LOCK = threading.Lock()
_CACHED = {}


def _declare_io(nc):
    io = {}

    def inp(name, shape):
        io[name] = nc.dram_tensor(name, shape, F32, kind="ExternalInput").ap()

    def outp(name, shape):
        io[name] = nc.dram_tensor(name, shape, F32, kind="ExternalOutput").ap()

    inp("xT", (NHID, BPC))
    inp("hxT", (NHID, BPC))
    inp("cxT", (NHID, BPC))
    inp("wq", (BS_OUT, DK_IN))
    inp("wk", (BS_IN, DK_IN))
    inp("wv", (BS_IN, ATT_OUT))
    inp("bv", (ATT_OUT,))
    inp("wihT", (8, 512, 512))
    inp("whhT", (8, 128, 512))
    inp("biasgT", (8, 4, 128))
    inp("bqi", (DK_IN,))
    inp("e16", (16, 16, 128))
    for nm in ("wqc", "wkc", "wvc", "wfc", "wgc"):
        inp(nm, (128, 128))
    for nm in ("bqc", "bkc", "bvc", "bfc", "bgc"):
        inp(nm, (128,))
    outp("hx_outT", (NHID, BPC))
    outp("cx_outT", (NHID, BPC))
    outp("mask_out", (BPC, NHID))
    outp("bm_out", (BPC, 8))
    return io


def _build_nc(use_bias):
    key = bool(use_bias)
    with _BUILD_LOCK:
        if key in _CACHED:
            return _CACHED[key]
        _apply_tile_patch()
        nc = bass.Bass("TRN2")
        io = _declare_io(nc)
        with tile.TileContext(nc) as tc:
            with ExitStack() as ctx:
                build_kernel(ctx, tc, io, use_bias)
        _CACHED[key] = nc
        return nc


def _pack_weights(inputs):
    W_ih = np.asarray(inputs["W_ih"], np.float32)
    W_hh = np.asarray(inputs["W_hh"], np.float32)
    b_ih = np.asarray(inputs["b_ih"], np.float32)
    b_hh = np.asarray(inputs["b_hh"], np.float32)
    wihT = np.empty((8, 512, 512), np.float32)
    whhT = np.empty((8, 128, 512), np.float32)
    biasg = np.empty((8, 4, 128), np.float32)
    for j in range(8):
        for g in range(4):
            rows = slice(g * NHID + j * 128, g * NHID + (j + 1) * 128)
            wihT[j][:, g * 128:(g + 1) * 128] = \
                W_ih[rows, j * 512:(j + 1) * 512].T
            whhT[j][:, g * 128:(g + 1) * 128] = \
                W_hh[rows, j * 128:(j + 1) * 128].T
            biasg[j, g] = (b_ih + b_hh)[rows]
    return wihT, whhT, biasg


def _cf(x):
    return np.ascontiguousarray(np.asarray(x, np.float32))


def _make_e16():
    e = np.zeros((16, 16, 128), np.float32)
    for r in range(16):
        e[r, r, :] = 1.0
    return e


def make_in_maps(inputs):
    inp = _cf(inputs["inp"])
    hx = _cf(inputs["hx"])
    cx = _cf(inputs["cx"])

    wihT, whhT, biasg = _pack_weights(inputs)
    bias_arrs = [inputs[k] for k in
                 ("bq_i", "bk_i", "bv_i", "bq_c", "bk_c", "bv_c",
                  "bf_c", "bg_c", "b_ih", "b_hh")]
    use_bias = bool(any(np.any(np.asarray(a)) for a in bias_arrs))

    shared = {
        "wq": _cf(inputs["Wq_i"]), "wk": _cf(inputs["Wk_i"]),
        "wv": _cf(inputs["Wv_i"]), "bv": _cf(inputs["bv_i"]),
        "wihT": wihT, "whhT": whhT, "biasgT": biasg,
        "bqi": _cf(inputs["bq_i"]),
        "e16": _make_e16(),
        "wqc": _cf(inputs["Wq_c"]), "wkc": _cf(inputs["Wk_c"]),
        "wvc": _cf(inputs["Wv_c"]),
        "bqc": _cf(inputs["bq_c"]), "bkc": _cf(inputs["bk_c"]),
        "bvc": _cf(inputs["bv_c"]),
        "wfc": _cf(inputs["Wf_c"]), "wgc": _cf(inputs["Wg_c"]),
        "bfc": _cf(inputs["bf_c"]), "bgc": _cf(inputs["bg_c"]),
    }

    in_maps = []
    for r in range(NCORES):
        rows = slice(r * BPC, (r + 1) * BPC)
        m = dict(shared)
        m["xT"] = np.ascontiguousarray(inp[rows].T)
        m["hxT"] = np.ascontiguousarray(hx[rows].T)
        m["cxT"] = np.ascontiguousarray(cx[rows].T)
        in_maps.append(m)
    return in_maps, use_bias


def assemble_outputs(results):
    hx_out = np.empty((B, NHID), np.float32)
    cx_out = np.empty((B, NHID), np.float32)
    mask = np.empty((B, NHID), np.float32)
    bm = np.empty((B, 8), np.float32)
    for r in range(NCORES):
        rows = slice(r * BPC, (r + 1) * BPC)
        out = results[r]
        hx_out[rows] = out["hx_outT"].T
        cx_out[rows] = out["cx_outT"].T
        mask[rows] = out["mask_out"]
        bm[rows] = out["bm_out"]
    return hx_out, cx_out, mask, bm.reshape(B, 8, 1)


def kernel(**inputs):
    in_maps, use_bias = make_in_maps(inputs)
    nc = _build_nc(use_bias)
    res = run_bass_kernel_spmd(nc, in_maps, core_ids=list(range(NCORES)))
    return assemble_outputs(res.results)


# revision 41
# speedup vs baseline: 1.0899x; 1.0899x over previous
"""Trainium2 Bass kernel for nn_BlocksCore (RIMs BlocksCore forward).

Data-parallel over batch across 8 NeuronCores (512 samples/core).
Compute layout: feature-major [feat, batch] for matmuls (weights stationary),
sample-major [batch, feat] for the LSTM cell / top-k gating, where per-sample
scalars (attention mixing weight, block mask) map to [P,1] tensor-scalar ops.

Numerical strategy: the input-attention score path (q, k, q.dk reduction) runs
in full fp32 because the top-4/bottom-4 block ranking has a min margin of
~1.8e-4 over the 4096 samples (fp32r matmuls, ~1.6e-4 relative error, would
flip masks). The dominant LSTM-gate matmuls run in fp32r (full PE rate at
N=512); fp32r operands must be produced as fp32r (walrus dataflow check), so
the hx stationary operand gets a one-time rounded copy.

Input attention is algebraically reduced: with 2 key blocks, softmax weights
are (1-a, a) with a = sigmoid(e1 - e0), so inp_use = v0 + a*(v1 - v0) and the
block-diagonal LSTM input projection becomes
    gates_ih_j = v0 @ WihT_j + (a_j * dv) @ WihT_j,
where the per-sample scale a_j is applied to dv's stationary-operand columns
(samples) via a PE row-select broadcast matmul. Everything accumulates into a
single PSUM tile per (block, sample-block).
"""

import threading
from contextlib import ExitStack

import numpy as np

import concourse.bass as bass
import concourse.mybir as mybir
import concourse.tile as tile
from concourse.bass_utils import run_bass_kernel_spmd
from concourse.masks import make_identity
from concourse.vector_clock import ScopedClock

F32 = mybir.dt.float32
F32R = mybir.dt.float32r
BF16 = mybir.dt.bfloat16
ALU = mybir.AluOpType
ACTF = mybir.ActivationFunctionType
AX = mybir.AxisListType

B = 4096
NCORES = 8
BPC = B // NCORES            # 512 samples per core
NSB = BPC // 128             # 4 sample blocks of 128
NHID = 1024
BS_IN = 512
BS_OUT = 128
ATT_OUT = 512
DK_IN = 64
INV_SQRT_DK_IN = 1.0 / 8.0
INV_SQRT_DK_C = float(1.0 / np.sqrt(32.0))


# ---------------------------------------------------------------------------
# Workarounds: this walrus build accepts at most ONE semaphore wait per
# instruction. (1) split the Tile tail-drain's waits across sequential SP
# drains; (2) after scheduling, hoist extra waits onto same-engine NOPs.
# ---------------------------------------------------------------------------
def _patched_drain_and_barrier(self, tick_clock, wait_clock):
    nc = self.nc
    drain_inst = nc.sync.drain()
    wait_clock.add_sem_waits(
        drain_inst.ins, ScopedClock({None: tick_clock.global_clock})
    )
    si = drain_inst.ins.sync_info
    if si is not None and si.on_wait is not None and len(si.on_wait) > 1:
        waits = list(si.on_wait)
        drain_inst.ins.sync_info = mybir.SyncInfo(
            on_wait=waits[:1], on_update=list(si.on_update or [])
        )
        for w in waits[1:]:
            d2 = nc.sync.drain()
            d2.ins.sync_info = mybir.SyncInfo(on_wait=[w], on_update=[])

    nc.all_engine_barrier()
    assert self.sems is not None
    popped = nc._tile_sem_poison_stack.pop()
    assert popped is self._sem_poison
    nc.clear_and_free_semaphores(list(self.sems.allocated().values()))
    nc.all_engine_barrier()


_ORIG_LOWER = tile.TileContext._lower_ordered_insts
_NOPID = [0]


def _split_multiwait_lower(self, ordered):
    for bb in list(ordered.keys()):
        out = []
        for inst in ordered[bb]:
            si = getattr(inst, "sync_info", None)
            if si is not None and si.on_wait is not None and len(si.on_wait) > 1:
                waits = list(si.on_wait)
                for w in waits[:-1]:
                    _NOPID[0] += 1
                    out.append(mybir.InstNoOp(
                        name=f"{inst.name}_mw{_NOPID[0]}",
                        sync_info=mybir.SyncInfo(on_wait=[w], on_update=[]),
                        bass_nofuse=True,
                        engine=inst.engine,
                    ))
                inst.sync_info = mybir.SyncInfo(
                    on_wait=[waits[-1]], on_update=list(si.on_update or []))
            out.append(inst)
        ordered[bb] = out
    return _ORIG_LOWER(self, ordered)


def _apply_tile_patch():
    tile.TileContext._drain_and_barrier = _patched_drain_and_barrier
    tile.TileContext._lower_ordered_insts = _split_multiwait_lower


def _r(ap):
    return ap.bitcast(F32R)


# ---------------------------------------------------------------------------
# Device kernel body
# ---------------------------------------------------------------------------
def build_kernel(ctx, tc, io, use_bias):
    nc = tc.nc

    consts = ctx.enter_context(tc.tile_pool(name="consts", bufs=1))
    acts = ctx.enter_context(tc.tile_pool(name="acts", bufs=1))
    wstream = ctx.enter_context(tc.tile_pool(name="wstream", bufs=2))
    tbig = ctx.enter_context(tc.tile_pool(name="tbig", bufs=1))
    tsmall = ctx.enter_context(tc.tile_pool(name="tsmall", bufs=2))
    cxp = ctx.enter_context(tc.tile_pool(name="cxp", bufs=3))
    dvsp = ctx.enter_context(tc.tile_pool(name="dvsp", bufs=4))
    qkvp = ctx.enter_context(tc.tile_pool(name="qkvp", bufs=1))
    outp = ctx.enter_context(tc.tile_pool(name="outp", bufs=3))
    mout = ctx.enter_context(tc.tile_pool(name="mout", bufs=1))
    ps_gate = ctx.enter_context(
        tc.tile_pool(name="ps_gate", bufs=2, space="PSUM"))
    ps_big = ctx.enter_context(tc.tile_pool(name="ps_big", bufs=2, space="PSUM"))
    ps_small = ctx.enter_context(
        tc.tile_pool(name="ps_small", bufs=3, space="PSUM"))
    ps_tr = ctx.enter_context(tc.tile_pool(name="ps_tr", bufs=1, space="PSUM"))

    # ---- constants -------------------------------------------------------
    ident = consts.tile([128, 128], F32, tag="ident")
    make_identity(nc, ident)
    identb = consts.tile([128, 128], BF16, tag="identb")
    make_identity(nc, identb)

    ones_col = consts.tile([1, 128], F32, tag="ones_col")
    nc.vector.memset(ones_col, 1.0)

    # host-provided row-select matrices (bigE[:, r, :] is [16, 128] with row
    # r all-ones): a matmul with it as lhsT broadcasts row r of a [16, N] rhs
    # across 128 output partitions.
    bigE = consts.tile([16, 16, 128], F32R, tag="bigE")
    nc.sync.dma_start(out=bigE, in_=io["e16"].bitcast(F32R))

    # lower-triangular [j, i] -> 1.0 iff i < j  (tie-break mask for top-k)
    iot_i = consts.tile([128, 8, 8], F32, tag="iot_i")
    iot_j = consts.tile([128, 8, 8], F32, tag="iot_j")
    nc.gpsimd.iota(iot_i, pattern=[[0, 8], [1, 8]], base=0,
                   channel_multiplier=0, allow_small_or_imprecise_dtypes=True)
    nc.gpsimd.iota(iot_j, pattern=[[1, 8], [0, 8]], base=0,
                   channel_multiplier=0, allow_small_or_imprecise_dtypes=True)
    lt8 = consts.tile([128, 8, 8], F32, tag="lt8")
    nc.vector.tensor_tensor(lt8, iot_i, iot_j, ALU.is_lt)

    # ---- load activations ------------------------------------------------
    xT = []
    for c in range(8):
        t = acts.tile([128, BPC], F32, tag=f"xT{c}")
        nc.sync.dma_start(out=t, in_=io["xT"][c * 128:(c + 1) * 128, :])
        xT.append(t)
    hxT = []
    for c in range(8):
        t = acts.tile([128, BPC], F32, tag=f"hxT{c}")
        nc.sync.dma_start(out=t, in_=io["hxT"][c * 128:(c + 1) * 128, :])
        hxT.append(t)

    # ---- load weights ----------------------------------------------------
    wq = consts.tile([128, DK_IN], F32, tag="wq")
    nc.sync.dma_start(out=wq, in_=io["wq"][:])
    wk = consts.tile([128, 4, DK_IN], F32, tag="wk")
    nc.sync.dma_start(out=wk, in_=io["wk"].rearrange("(c p) d -> p c d", p=128))
    wv = consts.tile([128, 4, ATT_OUT], F32, tag="wv")
    nc.sync.dma_start(out=wv, in_=io["wv"].rearrange("(c p) d -> p c d", p=128))
    bvt = consts.tile([128, 4], F32, tag="bvt")
    nc.sync.dma_start(out=bvt, in_=io["bv"].rearrange("(c p) -> p c", p=128))

    wc = {}
    for nm in ("wqc", "wkc", "wvc", "wfc", "wgc"):
        t = consts.tile([128, 128], F32, tag=nm)
        nc.sync.dma_start(out=t, in_=io[nm][:])
        wc[nm] = t

    # ---- dx = x1 - x0 ----------------------------------------------------
    dxT = []
    for c in range(4):
        t = acts.tile([128, BPC], F32, tag=f"dxT{c}")
        nc.vector.tensor_tensor(t, xT[4 + c], xT[c], ALU.subtract)
        dxT.append(t)

    # ---- scores, iatt1, mask (sample-major, full fp32) ------------------
    # mi16[sb][:, 0:8] = block mask, mi16[sb][:, 8:16] = iatt1
    mi16 = []
    for sb in range(NSB):
        sbs = slice(sb * 128, (sb + 1) * 128)
        ps_q = ps_big.tile([128, 512], F32, tag="big", name=f"psq{sb}")
        for j in range(8):
            nc.tensor.matmul(ps_q[:, j * 64:(j + 1) * 64], hxT[j][:, sbs], wq,
                             start=True, stop=(not use_bias))
            if use_bias:
                brow = tsmall.tile([1, 64], F32, tag="bias_row",
                                   name=f"bqi{j}_{sb}")
                nc.sync.dma_start(out=brow, in_=io["bqi"][None, :])
                nc.tensor.matmul(ps_q[:, j * 64:(j + 1) * 64], ones_col, brow,
                                 start=False, stop=True)
        ps_k = ps_small.tile([128, 64], F32, tag="cmm", name=f"psk{sb}")
        for c in range(4):
            nc.tensor.matmul(ps_k, dxT[c][:, sbs], wk[:, c, :],
                             start=(c == 0), stop=(c == 3))
        # ndk = k1 - k0, so s' = q . ndk / 8 = e1 - e0 (negated score)
        dk = tsmall.tile([128, 64], F32, tag="dk", name=f"dk{sb}")
        nc.scalar.copy(dk, ps_k)

        s_sb = tsmall.tile([128, 8], F32, tag="s_sb", name=f"s{sb}")
        junk = tsmall.tile([128, 64], F32, tag="junk", name=f"junk{sb}")
        for j in range(8):
            nc.vector.scalar_tensor_tensor(
                junk, ps_q[:, j * 64:(j + 1) * 64], INV_SQRT_DK_IN, dk,
                ALU.mult, ALU.mult, accum_out=s_sb[:, j:j + 1])

        mi = acts.tile([128, 16], F32, tag=f"mi16_{sb}")
        # s' = e1 - e0, so iatt1 = sigmoid(s')
        nc.scalar.activation(mi[:, 8:16], s_sb, ACTF.Sigmoid)

        # s' = -s: bottom-4 of s are the top-4 of s'. rank'_j =
        # #{i: s'_i > s'_j} + #{i<j: s'_i == s'_j}; keep rank' >= 4
        pm = tsmall.tile([128, 8, 8], F32, tag="pm", name=f"pm{sb}")
        pe = tsmall.tile([128, 8, 8], F32, tag="pe", name=f"pe{sb}")
        s_bi = s_sb[:, None, :].to_broadcast([128, 8, 8])   # s_i along inner
        s_bj = s_sb[:, :, None].to_broadcast([128, 8, 8])   # s_j along outer
        nc.vector.tensor_tensor(pm, s_bi, s_bj, ALU.is_gt)
        nc.vector.tensor_tensor(pe, s_bi, s_bj, ALU.is_equal)
        nc.vector.tensor_tensor(pe, pe, lt8, ALU.mult)
        nc.vector.tensor_tensor(pm, pm, pe, ALU.add)
        cnt = tsmall.tile([128, 8], F32, tag="cnt", name=f"cnt{sb}")
        nc.vector.reduce_sum(cnt, pm, axis=AX.X)
        nc.vector.tensor_scalar(mi[:, 0:8], cnt, 4.0, None, ALU.is_ge)
        mi16.append(mi)

        mbc = mout.tile([128, NHID], F32, tag="mbc", name=f"mbc{sb}")
        nc.vector.tensor_copy(mbc,
                              mi[:, 0:8, None].to_broadcast([128, 8, 128]))
        nc.sync.dma_start(out=io["mask_out"][sbs, :], in_=mbc)
        nc.sync.dma_start(out=io["bm_out"][sbs, :], in_=mi[:, 0:8])

    # hx stationary operand for the fp32r gate matmuls must be produced as
    # fp32r: one-time rounded copies (score path above used full-fp32 hxT)
    hxTr = []
    for c in range(8):
        t = acts.tile([128, BPC], F32R, tag=f"hxTr{c}")
        nc.vector.tensor_copy(t, hxT[c])
        hxTr.append(t)

    # miT: feature-major [16, BPC]; row j = mask_j, row 8+j = iatt1_j
    miT = acts.tile([16, BPC], F32R, tag="miT")
    for sb in range(NSB):
        pst = ps_tr.tile([128, 128], F32, tag="tr", name=f"mtr{sb}")
        nc.tensor.transpose(pst[0:16, :], mi16[sb], ident)
        nc.scalar.copy(miT[:, sb * 128:(sb + 1) * 128], pst[0:16, :])

    # ---- v0T, dvT (full fp32 matmuls; x/dx stay fp32 regions) -----------
    v0T, dvT = [], []
    for m in range(4):
        ps = ps_big.tile([128, BPC], F32, tag="big", name=f"psv0_{m}")
        for c in range(4):
            nc.tensor.matmul(ps, wv[:, c, m * 128:(m + 1) * 128],
                             xT[c], start=(c == 0), stop=(c == 3))
        t = acts.tile([128, BPC], F32R, tag=f"v0T{m}")
        nc.scalar.activation(t, ps, ACTF.Identity, bias=bvt[:, m:m + 1],
                             scale=1.0)
        v0T.append(t)
    for m in range(4):
        ps = ps_big.tile([128, BPC], F32, tag="big", name=f"psdv_{m}")
        for c in range(4):
            nc.tensor.matmul(ps, wv[:, c, m * 128:(m + 1) * 128],
                             dxT[c], start=(c == 0), stop=(c == 3))
        t = acts.tile([128, BPC], F32, tag=f"dvT{m}")
        nc.scalar.copy(t, ps)
        dvT.append(t)

    # ---- gates, LSTM cell, cx blend (feature-major) ---------------------
    # hbT reuses the xT slots (xT is dead after the v matmuls; both fp32)
    hbT = []
    for j in range(8):
        t = acts.tile([128, BPC], F32, tag=f"xT{j}")
        hbT.append(t)

    GATE_ACT = [ACTF.Sigmoid, ACTF.Sigmoid, ACTF.Tanh, ACTF.Sigmoid]
    for j in range(8):
        wih = wstream.tile([128, 4, 4, 128], F32R, tag="wih", name=f"wih{j}")
        nc.sync.dma_start(
            out=wih,
            in_=io["wihT"][j].rearrange("(c p) (gc go) -> p c gc go",
                                        p=128, go=128).bitcast(F32R))
        whh = wstream.tile([128, 4, 128], F32R, tag="whh", name=f"whh{j}")
        nc.sync.dma_start(
            out=whh,
            in_=io["whhT"][j].rearrange("p (gc go) -> p gc go",
                                        go=128).bitcast(F32R))
        cxTj = cxp.tile([128, BPC], F32, tag="cxT", name=f"cxT{j}")
        nc.sync.dma_start(out=cxTj, in_=io["cxT"][j * 128:(j + 1) * 128, :])

        # iatt1_j broadcast feature-major: bcA = row (8+j) of miT
        bcA = ps_big.tile([128, BPC], F32, tag="big", name=f"bcA{j}")
        nc.tensor.matmul(bcA, bigE[:, 8 + j, :], miT, start=True, stop=True)
        # dvs_c = iatt1_j * dvT_c  (scales the moving-operand columns)
        dvs = []
        for c in range(4):
            t = dvsp.tile([128, BPC], F32R, tag="dvs", name=f"dvs{j}_{c}")
            nc.vector.tensor_tensor(t, dvT[c], bcA, ALU.mult)
            dvs.append(t)

        # gates feature-major: one [128, BPC] tile per gate (i, f, g, o);
        # moving operand = activations (F32R), stationary = weight chunks
        gact = []
        for gc in range(4):
            psA = ps_gate.tile([128, BPC], F32, tag="psA", name=f"psA{j}_{gc}")
            for c in range(4):
                nc.tensor.matmul(psA, wih[:, c, gc, :], v0T[c],
                                 start=(c == 0), stop=False)
            for c in range(4):
                nc.tensor.matmul(psA, wih[:, c, gc, :], dvs[c],
                                 start=False, stop=False)
            nc.tensor.matmul(psA, whh[:, gc, :], hxTr[j],
                             start=False, stop=(not use_bias))
            if use_bias:
                bg_row = tsmall.tile([1, BPC], F32R, tag="bg_row",
                                     name=f"bg{j}_{gc}")
                nc.sync.dma_start(
                    out=bg_row,
                    in_=io["biasgT"][j, gc, :, None]
                    .to_broadcast([1, BPC]).bitcast(F32R))
                nc.tensor.matmul(psA, _r(ones_col), bg_row,
                                 start=False, stop=True)
            g = tsmall.tile([128, BPC], F32, tag=f"gact{gc}",
                            name=f"g{j}_{gc}")
            nc.scalar.activation(g, psA, GATE_ACT[gc])
            gact.append(g)

        sigi, sigf, tng, sgo = gact
        t1 = tbig.tile([128, BPC], F32, tag="t1", name=f"t1_{j}")
        nc.vector.tensor_tensor(t1, sigf, cxTj, ALU.mult)
        t2 = tbig.tile([128, BPC], F32, tag="t2", name=f"t2_{j}")
        nc.vector.tensor_tensor(t2, sigi, tng, ALU.mult)
        cxn = tbig.tile([128, BPC], F32, tag="cxn", name=f"cxn{j}")
        nc.vector.tensor_tensor(cxn, t1, t2, ALU.add)
        tnc = tbig.tile([128, BPC], F32, tag="tnc", name=f"tnc{j}")
        nc.scalar.activation(tnc, cxn, ACTF.Tanh)
        nc.vector.tensor_tensor(hbT[j], sgo, tnc, ALU.mult)

        # cx blend: cx_out = cx + mask_j * (cx_new - cx), feature-major
        mexpj = ps_small.tile([128, BPC], F32, tag="cmm", name=f"mexG{j}")
        nc.tensor.matmul(mexpj, bigE[:, j, :], miT, start=True, stop=True)
        nc.gpsimd.tensor_tensor(cxn, cxn, cxTj, ALU.subtract)
        dcm = tbig.tile([128, BPC], F32, tag="dcm", name=f"dcm{j}")
        nc.vector.tensor_tensor(dcm, cxn, mexpj, ALU.mult)
        cxo = outp.tile([128, BPC], F32, tag="cxo", name=f"cxo{j}")
        nc.gpsimd.tensor_tensor(cxo, dcm, cxTj, ALU.add)
        nc.sync.dma_start(out=io["cx_outT"][j * 128:(j + 1) * 128, :],
                          in_=cxo)

    # ---- communication attention + output fc + hx blend (per sb) --------
    for sb in range(NSB):
        sbs = slice(sb * 128, (sb + 1) * 128)
        qc, kc, vc = [], [], []
        for j in range(8):
            psq = ps_small.tile([128, 128], F32, tag="cmm",
                                name=f"pq{j}_{sb}")
            psk = ps_small.tile([128, 128], F32, tag="cmm",
                                name=f"pk{j}_{sb}")
            psv = ps_small.tile([128, 128], F32, tag="cmm",
                                name=f"pv{j}_{sb}")
            lhsT = hbT[j][:, sbs]
            nc.tensor.matmul(psq, lhsT, wc["wqc"],
                             start=True, stop=(not use_bias))
            nc.tensor.matmul(psk, lhsT, wc["wkc"],
                             start=True, stop=(not use_bias))
            nc.tensor.matmul(psv, lhsT, wc["wvc"],
                             start=True, stop=(not use_bias))
            if use_bias:
                for ps, bn in ((psq, "bqc"), (psk, "bkc"), (psv, "bvc")):
                    brow = tsmall.tile([1, 128], F32, tag="brow",
                                       name=f"b{bn}{j}_{sb}")
                    nc.sync.dma_start(out=brow, in_=io[bn][None, :])
                    nc.tensor.matmul(ps, ones_col, brow, start=False,
                                     stop=True)
            if j == 0:
                qcall = qkvp.tile([128, 8, 4, 32], BF16, tag="qcall",
                                  name=f"qcall{sb}")
            tk = qkvp.tile([128, 4, 32], BF16, tag=f"kc{j}")
            tv = qkvp.tile([128, 4, 32], BF16, tag=f"vc{j}")
            nc.scalar.copy(qcall[:, j], psq.rearrange("p (h d) -> p h d", d=32))
            nc.scalar.copy(tk, psk.rearrange("p (h d) -> p h d", d=32))
            nc.scalar.copy(tv, psv.rearrange("p (h d) -> p h d", d=32))
            kc.append(tk)
            vc.append(tv)

        # scores S[b, h, qi, ki]: batched over qi per ki; muls split
        # across DVE and GPSIMD, segmented reduces on DVE
        S3 = tbig.tile([128, 4, 64], BF16, tag="S3", name=f"S3_{sb}")
        S3r = S3.rearrange("p h (q k) -> p q h k", k=8)
        for ki in range(8):
            prodq = tbig.tile([128, 8, 4, 32], BF16, tag=f"prodq{ki % 2}",
                              name=f"prod{sb}_{ki}")
            k_bc = kc[ki][:, None, :, :].to_broadcast([128, 8, 4, 32])
            eng = nc.vector if ki % 2 == 0 else nc.gpsimd
            eng.tensor_tensor(prodq, qcall, k_bc, ALU.mult)
            nc.vector.reduce_sum(S3r[:, :, :, ki], prodq, axis=AX.X)
        # softmax over ki (exp and normalize in place)
        A = S3.rearrange("p h (q k) -> p h q k", k=8)
        nc.scalar.activation(A, A, ACTF.Exp, scale=INV_SQRT_DK_C)
        den = tsmall.tile([128, 4, 8], F32, tag="den", name=f"den{sb}")
        nc.vector.reduce_sum(den, A, axis=AX.X)
        rec = tsmall.tile([128, 4, 8], F32, tag="rec", name=f"rec{sb}")
        nc.vector.reciprocal(rec, den)
        nc.vector.tensor_tensor(
            A, A, rec[:, :, :, None].to_broadcast([128, 4, 8, 8]), ALU.mult)

        # AV: o[b, qi, h, d] = sum_ki A[b,h,qi,ki] * vc[b,ki,(h,d)]
        o_a = tbig.tile([128, 8, 4, 32], BF16, tag="o_a", name=f"oa{sb}")
        o_b = tbig.tile([128, 8, 4, 32], BF16, tag="o_b", name=f"ob{sb}")
        cur = o_a
        for ki in range(8):
            prod2 = tbig.tile([128, 8, 4, 32], BF16, tag=f"prodq{ki % 2}",
                              name=f"p2_{sb}_{ki}")
            a_sl = (A[:, :, :, ki]                   # [128, h, qi]
                    .rearrange("p h q -> p q h")     # [128, qi, h]
                    [:, :, :, None].to_broadcast([128, 8, 4, 32]))
            v_bc = vc[ki][:, None, :, :].to_broadcast([128, 8, 4, 32])
            eng = nc.vector if ki % 2 == 0 else nc.gpsimd
            if ki == 0:
                eng.tensor_tensor(cur, v_bc, a_sl, ALU.mult)
            else:
                eng.tensor_tensor(prod2, v_bc, a_sl, ALU.mult)
                nxt = o_b if cur is o_a else o_a
                nc.vector.tensor_tensor(nxt, cur, prod2, ALU.add)
                cur = nxt

        # per block: transpose o, output fc, gated tanh, hx blend
        for j in range(8):
            pst = ps_tr.tile([128, 128], BF16, tag="tr", name=f"otr{j}_{sb}")
            nc.tensor.transpose(pst, cur[:, j], identb)
            otmp = tsmall.tile([128, 128], F32, tag="otmp",
                               name=f"ot{j}_{sb}")
            nc.scalar.copy(otmp, pst)

            psf = ps_small.tile([128, 128], F32, tag="cmm", name=f"psf{j}_{sb}")
            psg = ps_small.tile([128, 128], F32, tag="cmm", name=f"psg{j}_{sb}")
            nc.tensor.matmul(psf, wc["wfc"], otmp, start=True, stop=True)
            nc.tensor.matmul(psg, wc["wgc"], otmp, start=True, stop=True)
            tf = tsmall.tile([128, 128], F32, tag="tf", name=f"tf{j}_{sb}")
            sg = tsmall.tile([128, 128], F32, tag="sg", name=f"sg{j}_{sb}")
            if use_bias:
                bfcol = consts.tile([128, 1], F32, tag="bfcol")
                bgcol = consts.tile([128, 1], F32, tag="bgcol")
                if j == 0 and sb == 0:
                    nc.sync.dma_start(out=bfcol, in_=io["bfc"][:, None])
                    nc.sync.dma_start(out=bgcol, in_=io["bgc"][:, None])
                nc.scalar.activation(tf, psf, ACTF.Tanh, bias=bfcol, scale=1.0)
                nc.scalar.activation(sg, psg, ACTF.Sigmoid, bias=bgcol,
                                     scale=1.0)
            else:
                nc.scalar.activation(tf, psf, ACTF.Tanh)
                nc.scalar.activation(sg, psg, ACTF.Sigmoid)
            # comm = sigmoid(og) * tanh(of), in place on sg
            nc.vector.tensor_tensor(sg, sg, tf, ALU.mult)

            # hx_new = hb + comm; hx_out = hx + mask*(hx_new - hx)
            mexp = ps_small.tile([128, 128], F32, tag="cmm",
                                name=f"mexp{j}_{sb}")
            nc.tensor.matmul(mexp, bigE[:, j, :], miT[:, sbs],
                             start=True, stop=True)
            hb = hbT[j][:, sbs]
            nc.vector.tensor_tensor(hb, hb, sg, ALU.add)
            nc.vector.tensor_tensor(hb, hb, hxT[j][:, sbs], ALU.subtract)
            nc.vector.tensor_tensor(hb, hb, mexp, ALU.mult)
            hout = outp.tile([128, 128], F32, tag="hout", name=f"ho{j}_{sb}")
            nc.vector.tensor_tensor(hout, hb, hxT[j][:, sbs], ALU.add)
            nc.sync.dma_start(out=io["hx_outT"][j * 128:(j + 1) * 128, sbs],
                              in_=hout)


# ---------------------------------------------------------------------------
# Host side
# ---------------------------------------------------------------------------
_BUILD_LOCK = threading.Lock()
_CACHED = {}


def _declare_io(nc):
    io = {}

    def inp(name, shape):
        io[name] = nc.dram_tensor(name, shape, F32, kind="ExternalInput").ap()

    def outp(name, shape):
        io[name] = nc.dram_tensor(name, shape, F32, kind="ExternalOutput").ap()

    inp("xT", (NHID, BPC))
    inp("hxT", (NHID, BPC))
    inp("cxT", (NHID, BPC))
    inp("wq", (BS_OUT, DK_IN))
    inp("wk", (BS_IN, DK_IN))
    inp("wv", (BS_IN, ATT_OUT))
    inp("bv", (ATT_OUT,))
    inp("wihT", (8, 512, 512))
    inp("whhT", (8, 128, 512))
    inp("biasgT", (8, 4, 128))
    inp("bqi", (DK_IN,))
    inp("e16", (16, 16, 128))
    for nm in ("wqc", "wkc", "wvc", "wfc", "wgc"):
        inp(nm, (128, 128))
    for nm in ("bqc", "bkc", "bvc", "bfc", "bgc"):
        inp(nm, (128,))
    outp("hx_outT", (NHID, BPC))
    outp("cx_outT", (NHID, BPC))
    outp("mask_out", (BPC, NHID))
    outp("bm_out", (BPC, 8))
    return io


def _build_nc(use_bias):
    key = bool(use_bias)
    with _BUILD_LOCK:
        if key in _CACHED:
            return _CACHED[key]
        _apply_tile_patch()
        nc = bass.Bass("TRN2")
        io = _declare_io(nc)
        with nc.allow_low_precision("comm attention core in bf16; "
                                     "not on the top-k mask path"):
            with tile.TileContext(nc) as tc:
                with ExitStack() as ctx:
                    build_kernel(ctx, tc, io, use_bias)
        _CACHED[key] = nc
        return nc


def _pack_weights(inputs):
    W_ih = np.asarray(inputs["W_ih"], np.float32)
    W_hh = np.asarray(inputs["W_hh"], np.float32)
    b_ih = np.asarray(inputs["b_ih"], np.float32)
    b_hh = np.asarray(inputs["b_hh"], np.float32)
    wihT = np.empty((8, 512, 512), np.float32)
    whhT = np.empty((8, 128, 512), np.float32)
    biasg = np.empty((8, 4, 128), np.float32)
    for j in range(8):
        for g in range(4):
            rows = slice(g * NHID + j * 128, g * NHID + (j + 1) * 128)
            wihT[j][:, g * 128:(g + 1) * 128] = \
                W_ih[rows, j * 512:(j + 1) * 512].T
            whhT[j][:, g * 128:(g + 1) * 128] = \
                W_hh[rows, j * 128:(j + 1) * 128].T
            biasg[j, g] = (b_ih + b_hh)[rows]
    return wihT, whhT, biasg


def _cf(x):
    return np.ascontiguousarray(np.asarray(x, np.float32))


def _make_e16():
    e = np.zeros((16, 16, 128), np.float32)
    for r in range(16):
        e[r, r, :] = 1.0
    return e


def make_in_maps(inputs):
    inp = _cf(inputs["inp"])
    hx = _cf(inputs["hx"])
    cx = _cf(inputs["cx"])

    wihT, whhT, biasg = _pack_weights(inputs)
    bias_arrs = [inputs[k] for k in
                 ("bq_i", "bk_i", "bv_i", "bq_c", "bk_c", "bv_c",
                  "bf_c", "bg_c", "b_ih", "b_hh")]
    use_bias = bool(any(np.any(np.asarray(a)) for a in bias_arrs))

    shared = {
        "wq": _cf(inputs["Wq_i"]), "wk": _cf(inputs["Wk_i"]),
        "wv": _cf(inputs["Wv_i"]), "bv": _cf(inputs["bv_i"]),
        "wihT": wihT, "whhT": whhT, "biasgT": biasg,
        "bqi": _cf(inputs["bq_i"]),
        "e16": _make_e16(),
        "wqc": _cf(inputs["Wq_c"]), "wkc": _cf(inputs["Wk_c"]),
        "wvc": _cf(inputs["Wv_c"]),
        "bqc": _cf(inputs["bq_c"]), "bkc": _cf(inputs["bk_c"]),
        "bvc": _cf(inputs["bv_c"]),
        "wfc": _cf(inputs["Wf_c"]), "wgc": _cf(inputs["Wg_c"]),
        "bfc": _cf(inputs["bf_c"]), "bgc": _cf(inputs["bg_c"]),
    }

    in_maps = []
    for r in range(NCORES):
        rows = slice(r * BPC, (r + 1) * BPC)
        m = dict(shared)
        m["xT"] = np.ascontiguousarray(inp[rows].T)
        m["hxT"] = np.ascontiguousarray(hx[rows].T)
        m["cxT"] = np.ascontiguousarray(cx[rows].T)
        in_maps.append(m)
    return in_maps, use_bias


def assemble_outputs(results):
    hx_out = np.empty((B, NHID), np.float32)
    cx_out = np.empty((B, NHID), np.float32)
    mask = np.empty((B, NHID), np.float32)
    bm = np.empty((B, 8), np.float32)
    for r in range(NCORES):
        rows = slice(r * BPC, (r + 1) * BPC)
        out = results[r]
        hx_out[rows] = out["hx_outT"].T
        cx_out[rows] = out["cx_outT"].T
        mask[rows] = out["mask_out"]
        bm[rows] = out["bm_out"]
    return hx_out, cx_out, mask, bm.reshape(B, 8, 1)


def kernel(**inputs):
    in_maps, use_bias = make_in_maps(inputs)
    nc = _build_nc(use_bias)
    res = run_bass_kernel_spmd(nc, in_maps, core_ids=list(range(NCORES)))
    return assemble_outputs(res.results)


# revision 44
# speedup vs baseline: 1.1521x; 1.0571x over previous
"""Trainium2 Bass kernel for nn_BlocksCore (RIMs BlocksCore forward).

Data-parallel over batch across 8 NeuronCores (512 samples/core).
Compute layout: feature-major [feat, batch] for matmuls (weights stationary),
sample-major [batch, feat] for the LSTM cell / top-k gating, where per-sample
scalars (attention mixing weight, block mask) map to [P,1] tensor-scalar ops.

Numerical strategy: the input-attention score path (q, k, q.dk reduction) runs
in full fp32 because the top-4/bottom-4 block ranking has a min margin of
~1.8e-4 over the 4096 samples (fp32r matmuls, ~1.6e-4 relative error, would
flip masks). The dominant LSTM-gate matmuls run in fp32r (full PE rate at
N=512); fp32r operands must be produced as fp32r (walrus dataflow check), so
the hx stationary operand gets a one-time rounded copy.

Input attention is algebraically reduced: with 2 key blocks, softmax weights
are (1-a, a) with a = sigmoid(e1 - e0), so inp_use = v0 + a*(v1 - v0) and the
block-diagonal LSTM input projection becomes
    gates_ih_j = v0 @ WihT_j + (a_j * dv) @ WihT_j,
where the per-sample scale a_j is applied to dv's stationary-operand columns
(samples) via a PE row-select broadcast matmul. Everything accumulates into a
single PSUM tile per (block, sample-block).
"""

import threading
from contextlib import ExitStack

import numpy as np

import concourse.bass as bass
import concourse.mybir as mybir
import concourse.tile as tile
from concourse.bass_utils import run_bass_kernel_spmd
from concourse.masks import make_identity
from concourse.vector_clock import ScopedClock

F32 = mybir.dt.float32
F32R = mybir.dt.float32r
BF16 = mybir.dt.bfloat16
ALU = mybir.AluOpType
ACTF = mybir.ActivationFunctionType
AX = mybir.AxisListType

B = 4096
NCORES = 8
BPC = B // NCORES            # 512 samples per core
NSB = BPC // 128             # 4 sample blocks of 128
NHID = 1024
BS_IN = 512
BS_OUT = 128
ATT_OUT = 512
DK_IN = 64
INV_SQRT_DK_IN = 1.0 / 8.0
INV_SQRT_DK_C = float(1.0 / np.sqrt(32.0))


# ---------------------------------------------------------------------------
# Workarounds: this walrus build accepts at most ONE semaphore wait per
# instruction. (1) split the Tile tail-drain's waits across sequential SP
# drains; (2) after scheduling, hoist extra waits onto same-engine NOPs.
# ---------------------------------------------------------------------------
def _patched_drain_and_barrier(self, tick_clock, wait_clock):
    nc = self.nc
    drain_inst = nc.sync.drain()
    wait_clock.add_sem_waits(
        drain_inst.ins, ScopedClock({None: tick_clock.global_clock})
    )
    si = drain_inst.ins.sync_info
    if si is not None and si.on_wait is not None and len(si.on_wait) > 1:
        waits = list(si.on_wait)
        drain_inst.ins.sync_info = mybir.SyncInfo(
            on_wait=waits[:1], on_update=list(si.on_update or [])
        )
        for w in waits[1:]:
            d2 = nc.sync.drain()
            d2.ins.sync_info = mybir.SyncInfo(on_wait=[w], on_update=[])

    nc.all_engine_barrier()
    assert self.sems is not None
    popped = nc._tile_sem_poison_stack.pop()
    assert popped is self._sem_poison
    nc.clear_and_free_semaphores(list(self.sems.allocated().values()))
    nc.all_engine_barrier()


_ORIG_LOWER = tile.TileContext._lower_ordered_insts
_NOPID = [0]


def _split_multiwait_lower(self, ordered):
    for bb in list(ordered.keys()):
        out = []
        for inst in ordered[bb]:
            si = getattr(inst, "sync_info", None)
            if si is not None and si.on_wait is not None and len(si.on_wait) > 1:
                waits = list(si.on_wait)
                for w in waits[:-1]:
                    _NOPID[0] += 1
                    out.append(mybir.InstNoOp(
                        name=f"{inst.name}_mw{_NOPID[0]}",
                        sync_info=mybir.SyncInfo(on_wait=[w], on_update=[]),
                        bass_nofuse=True,
                        engine=inst.engine,
                    ))
                inst.sync_info = mybir.SyncInfo(
                    on_wait=[waits[-1]], on_update=list(si.on_update or []))
            out.append(inst)
        ordered[bb] = out
    return _ORIG_LOWER(self, ordered)


def _apply_tile_patch():
    tile.TileContext._drain_and_barrier = _patched_drain_and_barrier
    tile.TileContext._lower_ordered_insts = _split_multiwait_lower


def _r(ap):
    return ap.bitcast(F32R)


# ---------------------------------------------------------------------------
# Device kernel body
# ---------------------------------------------------------------------------
def build_kernel(ctx, tc, io, use_bias):
    nc = tc.nc

    consts = ctx.enter_context(tc.tile_pool(name="consts", bufs=1))
    acts = ctx.enter_context(tc.tile_pool(name="acts", bufs=1))
    wstream = ctx.enter_context(tc.tile_pool(name="wstream", bufs=2))
    tbig = ctx.enter_context(tc.tile_pool(name="tbig", bufs=1))
    tsmall = ctx.enter_context(tc.tile_pool(name="tsmall", bufs=2))
    cxp = ctx.enter_context(tc.tile_pool(name="cxp", bufs=3))
    dvsp = ctx.enter_context(tc.tile_pool(name="dvsp", bufs=4))
    qkvp = ctx.enter_context(tc.tile_pool(name="qkvp", bufs=1))
    commp = ctx.enter_context(tc.tile_pool(name="commp", bufs=2))
    outp = ctx.enter_context(tc.tile_pool(name="outp", bufs=3))
    mout = ctx.enter_context(tc.tile_pool(name="mout", bufs=1))
    ps_gate = ctx.enter_context(
        tc.tile_pool(name="ps_gate", bufs=2, space="PSUM"))
    ps_big = ctx.enter_context(tc.tile_pool(name="ps_big", bufs=2, space="PSUM"))
    ps_small = ctx.enter_context(
        tc.tile_pool(name="ps_small", bufs=3, space="PSUM"))
    ps_tr = ctx.enter_context(tc.tile_pool(name="ps_tr", bufs=1, space="PSUM"))

    # ---- constants -------------------------------------------------------
    ident = consts.tile([128, 128], F32, tag="ident")
    make_identity(nc, ident)
    identb = consts.tile([128, 128], BF16, tag="identb")
    make_identity(nc, identb)

    ones_col = consts.tile([1, 128], F32, tag="ones_col")
    nc.vector.memset(ones_col, 1.0)

    # host-provided row-select matrices (bigE[:, r, :] is [16, 128] with row
    # r all-ones): a matmul with it as lhsT broadcasts row r of a [16, N] rhs
    # across 128 output partitions.
    bigE = consts.tile([16, 16, 128], F32R, tag="bigE")
    nc.sync.dma_start(out=bigE, in_=io["e16"].bitcast(F32R))

    # lower-triangular [j, i] -> 1.0 iff i < j  (tie-break mask for top-k)
    iot_i = consts.tile([128, 8, 8], F32, tag="iot_i")
    iot_j = consts.tile([128, 8, 8], F32, tag="iot_j")
    nc.gpsimd.iota(iot_i, pattern=[[0, 8], [1, 8]], base=0,
                   channel_multiplier=0, allow_small_or_imprecise_dtypes=True)
    nc.gpsimd.iota(iot_j, pattern=[[1, 8], [0, 8]], base=0,
                   channel_multiplier=0, allow_small_or_imprecise_dtypes=True)
    lt8 = consts.tile([128, 8, 8], F32, tag="lt8")
    nc.vector.tensor_tensor(lt8, iot_i, iot_j, ALU.is_lt)

    # ---- load activations ------------------------------------------------
    xT = []
    for c in range(8):
        t = acts.tile([128, BPC], F32, tag=f"xT{c}")
        nc.sync.dma_start(out=t, in_=io["xT"][c * 128:(c + 1) * 128, :])
        xT.append(t)
    hxT = []
    for c in range(8):
        t = acts.tile([128, BPC], F32, tag=f"hxT{c}")
        nc.sync.dma_start(out=t, in_=io["hxT"][c * 128:(c + 1) * 128, :])
        hxT.append(t)

    # ---- load weights ----------------------------------------------------
    wq = consts.tile([128, DK_IN], F32, tag="wq")
    nc.sync.dma_start(out=wq, in_=io["wq"][:])
    wk = consts.tile([128, 4, DK_IN], F32, tag="wk")
    nc.sync.dma_start(out=wk, in_=io["wk"].rearrange("(c p) d -> p c d", p=128))
    wv = consts.tile([128, 4, ATT_OUT], F32, tag="wv")
    nc.sync.dma_start(out=wv, in_=io["wv"].rearrange("(c p) d -> p c d", p=128))
    bvt = consts.tile([128, 4], F32, tag="bvt")
    nc.sync.dma_start(out=bvt, in_=io["bv"].rearrange("(c p) -> p c", p=128))

    wc = {}
    for nm in ("wqc", "wkc", "wvc", "wfc", "wgc"):
        t = consts.tile([128, 128], F32, tag=nm)
        nc.sync.dma_start(out=t, in_=io[nm][:])
        wc[nm] = t

    # ---- dx = x1 - x0 ----------------------------------------------------
    dxT = []
    for c in range(4):
        t = acts.tile([128, BPC], F32, tag=f"dxT{c}")
        nc.vector.tensor_tensor(t, xT[4 + c], xT[c], ALU.subtract)
        dxT.append(t)

    # ---- scores, iatt1, mask (sample-major, full fp32) ------------------
    # mi16[sb][:, 0:8] = block mask, mi16[sb][:, 8:16] = iatt1
    mi16 = []
    for sb in range(NSB):
        sbs = slice(sb * 128, (sb + 1) * 128)
        ps_q = ps_big.tile([128, 512], F32, tag="big", name=f"psq{sb}")
        for j in range(8):
            nc.tensor.matmul(ps_q[:, j * 64:(j + 1) * 64], hxT[j][:, sbs], wq,
                             start=True, stop=(not use_bias))
            if use_bias:
                brow = tsmall.tile([1, 64], F32, tag="bias_row",
                                   name=f"bqi{j}_{sb}")
                nc.sync.dma_start(out=brow, in_=io["bqi"][None, :])
                nc.tensor.matmul(ps_q[:, j * 64:(j + 1) * 64], ones_col, brow,
                                 start=False, stop=True)
        ps_k = ps_small.tile([128, 64], F32, tag="cmm", name=f"psk{sb}")
        for c in range(4):
            nc.tensor.matmul(ps_k, dxT[c][:, sbs], wk[:, c, :],
                             start=(c == 0), stop=(c == 3))
        # ndk = k1 - k0, so s' = q . ndk / 8 = e1 - e0 (negated score)
        dk = tsmall.tile([128, 64], F32, tag="dk", name=f"dk{sb}")
        nc.scalar.copy(dk, ps_k)

        s_sb = tsmall.tile([128, 8], F32, tag="s_sb", name=f"s{sb}")
        junk = tsmall.tile([128, 64], F32, tag="junk", name=f"junk{sb}")
        for j in range(8):
            nc.vector.scalar_tensor_tensor(
                junk, ps_q[:, j * 64:(j + 1) * 64], INV_SQRT_DK_IN, dk,
                ALU.mult, ALU.mult, accum_out=s_sb[:, j:j + 1])

        mi = acts.tile([128, 16], F32, tag=f"mi16_{sb}")
        # s' = e1 - e0, so iatt1 = sigmoid(s')
        nc.scalar.activation(mi[:, 8:16], s_sb, ACTF.Sigmoid)

        # s' = -s: bottom-4 of s are the top-4 of s'. rank'_j =
        # #{i: s'_i > s'_j} + #{i<j: s'_i == s'_j}; keep rank' >= 4
        pm = tsmall.tile([128, 8, 8], F32, tag="pm", name=f"pm{sb}")
        pe = tsmall.tile([128, 8, 8], F32, tag="pe", name=f"pe{sb}")
        s_bi = s_sb[:, None, :].to_broadcast([128, 8, 8])   # s_i along inner
        s_bj = s_sb[:, :, None].to_broadcast([128, 8, 8])   # s_j along outer
        nc.vector.tensor_tensor(pm, s_bi, s_bj, ALU.is_gt)
        nc.vector.tensor_tensor(pe, s_bi, s_bj, ALU.is_equal)
        nc.vector.tensor_tensor(pe, pe, lt8, ALU.mult)
        nc.vector.tensor_tensor(pm, pm, pe, ALU.add)
        cnt = tsmall.tile([128, 8], F32, tag="cnt", name=f"cnt{sb}")
        nc.vector.reduce_sum(cnt, pm, axis=AX.X)
        nc.vector.tensor_scalar(mi[:, 0:8], cnt, 4.0, None, ALU.is_ge)
        mi16.append(mi)

        mbc = mout.tile([128, NHID], F32, tag="mbc", name=f"mbc{sb}")
        nc.vector.tensor_copy(mbc,
                              mi[:, 0:8, None].to_broadcast([128, 8, 128]))
        nc.sync.dma_start(out=io["mask_out"][sbs, :], in_=mbc)
        nc.sync.dma_start(out=io["bm_out"][sbs, :], in_=mi[:, 0:8])

    # hx stationary operand for the fp32r gate matmuls must be produced as
    # fp32r: one-time rounded copies (score path above used full-fp32 hxT)
    hxTr = []
    for c in range(8):
        t = acts.tile([128, BPC], F32R, tag=f"hxTr{c}")
        nc.vector.tensor_copy(t, hxT[c])
        hxTr.append(t)

    # miT: feature-major [16, BPC]; row j = mask_j, row 8+j = iatt1_j
    miT = acts.tile([16, BPC], F32R, tag="miT")
    for sb in range(NSB):
        pst = ps_tr.tile([128, 128], F32, tag="tr", name=f"mtr{sb}")
        nc.tensor.transpose(pst[0:16, :], mi16[sb], ident)
        nc.scalar.copy(miT[:, sb * 128:(sb + 1) * 128], pst[0:16, :])

    # ---- v0T, dvT (full fp32 matmuls; x/dx stay fp32 regions) -----------
    v0T, dvT = [], []
    for m in range(4):
        ps = ps_big.tile([128, BPC], F32, tag="big", name=f"psv0_{m}")
        for c in range(4):
            nc.tensor.matmul(ps, wv[:, c, m * 128:(m + 1) * 128],
                             xT[c], start=(c == 0), stop=(c == 3))
        t = acts.tile([128, BPC], F32R, tag=f"v0T{m}")
        nc.scalar.activation(t, ps, ACTF.Identity, bias=bvt[:, m:m + 1],
                             scale=1.0)
        v0T.append(t)
    for m in range(4):
        ps = ps_big.tile([128, BPC], F32, tag="big", name=f"psdv_{m}")
        for c in range(4):
            nc.tensor.matmul(ps, wv[:, c, m * 128:(m + 1) * 128],
                             dxT[c], start=(c == 0), stop=(c == 3))
        t = acts.tile([128, BPC], F32, tag=f"dvT{m}")
        nc.scalar.copy(t, ps)
        dvT.append(t)

    # ---- gates, LSTM cell, cx blend (feature-major) ---------------------
    # hbT reuses the xT slots (xT is dead after the v matmuls; both fp32)
    hbT = []
    for j in range(8):
        t = acts.tile([128, BPC], F32, tag=f"xT{j}")
        hbT.append(t)

    GATE_ACT = [ACTF.Sigmoid, ACTF.Sigmoid, ACTF.Tanh, ACTF.Sigmoid]
    for j in range(8):
        wih = wstream.tile([128, 4, 4, 128], F32R, tag="wih", name=f"wih{j}")
        nc.sync.dma_start(
            out=wih,
            in_=io["wihT"][j].rearrange("(c p) (gc go) -> p c gc go",
                                        p=128, go=128).bitcast(F32R))
        whh = wstream.tile([128, 4, 128], F32R, tag="whh", name=f"whh{j}")
        nc.sync.dma_start(
            out=whh,
            in_=io["whhT"][j].rearrange("p (gc go) -> p gc go",
                                        go=128).bitcast(F32R))
        cxTj = cxp.tile([128, BPC], F32, tag="cxT", name=f"cxT{j}")
        nc.sync.dma_start(out=cxTj, in_=io["cxT"][j * 128:(j + 1) * 128, :])

        # iatt1_j broadcast feature-major: bcA = row (8+j) of miT
        bcA = ps_big.tile([128, BPC], F32, tag="big", name=f"bcA{j}")
        nc.tensor.matmul(bcA, bigE[:, 8 + j, :], miT, start=True, stop=True)
        # dvs_c = iatt1_j * dvT_c  (scales the moving-operand columns)
        dvs = []
        for c in range(4):
            t = dvsp.tile([128, BPC], F32R, tag="dvs", name=f"dvs{j}_{c}")
            nc.vector.tensor_tensor(t, dvT[c], bcA, ALU.mult)
            dvs.append(t)

        # gates feature-major: one [128, BPC] tile per gate (i, f, g, o);
        # moving operand = activations (F32R), stationary = weight chunks
        gact = []
        for gc in range(4):
            psA = ps_gate.tile([128, BPC], F32, tag="psA", name=f"psA{j}_{gc}")
            for c in range(4):
                nc.tensor.matmul(psA, wih[:, c, gc, :], v0T[c],
                                 start=(c == 0), stop=False)
            for c in range(4):
                nc.tensor.matmul(psA, wih[:, c, gc, :], dvs[c],
                                 start=False, stop=False)
            nc.tensor.matmul(psA, whh[:, gc, :], hxTr[j],
                             start=False, stop=(not use_bias))
            if use_bias:
                bg_row = tsmall.tile([1, BPC], F32R, tag="bg_row",
                                     name=f"bg{j}_{gc}")
                nc.sync.dma_start(
                    out=bg_row,
                    in_=io["biasgT"][j, gc, :, None]
                    .to_broadcast([1, BPC]).bitcast(F32R))
                nc.tensor.matmul(psA, _r(ones_col), bg_row,
                                 start=False, stop=True)
            g = tsmall.tile([128, BPC], F32, tag=f"gact{gc}",
                            name=f"g{j}_{gc}")
            nc.scalar.activation(g, psA, GATE_ACT[gc])
            gact.append(g)

        sigi, sigf, tng, sgo = gact
        t1 = tbig.tile([128, BPC], F32, tag="t1", name=f"t1_{j}")
        nc.vector.tensor_tensor(t1, sigf, cxTj, ALU.mult)
        t2 = tbig.tile([128, BPC], F32, tag="t2", name=f"t2_{j}")
        nc.vector.tensor_tensor(t2, sigi, tng, ALU.mult)
        cxn = tbig.tile([128, BPC], F32, tag="cxn", name=f"cxn{j}")
        nc.vector.tensor_tensor(cxn, t1, t2, ALU.add)
        tnc = tbig.tile([128, BPC], F32, tag="tnc", name=f"tnc{j}")
        nc.scalar.activation(tnc, cxn, ACTF.Tanh)
        nc.vector.tensor_tensor(hbT[j], sgo, tnc, ALU.mult)

        # cx blend: cx_out = cx + mask_j * (cx_new - cx), feature-major
        mexpj = ps_small.tile([128, BPC], F32, tag="cmm", name=f"mexG{j}")
        nc.tensor.matmul(mexpj, bigE[:, j, :], miT, start=True, stop=True)
        nc.gpsimd.tensor_tensor(cxn, cxn, cxTj, ALU.subtract)
        dcm = tbig.tile([128, BPC], F32, tag="dcm", name=f"dcm{j}")
        nc.vector.tensor_tensor(dcm, cxn, mexpj, ALU.mult)
        cxo = outp.tile([128, BPC], F32, tag="cxo", name=f"cxo{j}")
        nc.gpsimd.tensor_tensor(cxo, dcm, cxTj, ALU.add)
        nc.sync.dma_start(out=io["cx_outT"][j * 128:(j + 1) * 128, :],
                          in_=cxo)

    # ---- communication attention + output fc + hx blend (per sb) --------
    for sb in range(NSB):
        sbs = slice(sb * 128, (sb + 1) * 128)
        qc, kc, vc = [], [], []
        for j in range(8):
            psq = ps_small.tile([128, 128], F32, tag="cmm",
                                name=f"pq{j}_{sb}")
            psk = ps_small.tile([128, 128], F32, tag="cmm",
                                name=f"pk{j}_{sb}")
            psv = ps_small.tile([128, 128], F32, tag="cmm",
                                name=f"pv{j}_{sb}")
            lhsT = hbT[j][:, sbs]
            nc.tensor.matmul(psq, lhsT, wc["wqc"],
                             start=True, stop=(not use_bias))
            nc.tensor.matmul(psk, lhsT, wc["wkc"],
                             start=True, stop=(not use_bias))
            nc.tensor.matmul(psv, lhsT, wc["wvc"],
                             start=True, stop=(not use_bias))
            if use_bias:
                for ps, bn in ((psq, "bqc"), (psk, "bkc"), (psv, "bvc")):
                    brow = tsmall.tile([1, 128], F32, tag="brow",
                                       name=f"b{bn}{j}_{sb}")
                    nc.sync.dma_start(out=brow, in_=io[bn][None, :])
                    nc.tensor.matmul(ps, ones_col, brow, start=False,
                                     stop=True)
            if j == 0:
                qcall = qkvp.tile([128, 8, 4, 32], BF16, tag="qcall",
                                  name=f"qcall{sb}")
            tk = qkvp.tile([128, 4, 32], BF16, tag=f"kc{j}")
            tv = qkvp.tile([128, 4, 32], BF16, tag=f"vc{j}")
            nc.scalar.copy(qcall[:, j], psq.rearrange("p (h d) -> p h d", d=32))
            nc.scalar.copy(tk, psk.rearrange("p (h d) -> p h d", d=32))
            nc.scalar.copy(tv, psv.rearrange("p (h d) -> p h d", d=32))
            kc.append(tk)
            vc.append(tv)

        # scores S[b, h, qi, ki]: batched over qi per ki; muls split
        # across DVE and GPSIMD, segmented reduces on DVE
        S3 = commp.tile([128, 4, 64], BF16, tag="S3", name=f"S3_{sb}")
        S3r = S3.rearrange("p h (q k) -> p q h k", k=8)
        for ki in range(8):
            prodq = commp.tile([128, 8, 4, 32], BF16, tag=f"prodq{ki % 2}",
                              name=f"prod{sb}_{ki}")
            k_bc = kc[ki][:, None, :, :].to_broadcast([128, 8, 4, 32])
            eng = nc.vector if ki % 2 == 0 else nc.gpsimd
            eng.tensor_tensor(prodq, qcall, k_bc, ALU.mult)
            nc.vector.reduce_sum(S3r[:, :, :, ki], prodq, axis=AX.X)
        # softmax over ki (exp and normalize in place)
        A = S3.rearrange("p h (q k) -> p h q k", k=8)
        nc.scalar.activation(A, A, ACTF.Exp, scale=INV_SQRT_DK_C)
        den = tsmall.tile([128, 4, 8], F32, tag="den", name=f"den{sb}")
        nc.vector.reduce_sum(den, A, axis=AX.X)
        rec = tsmall.tile([128, 4, 8], F32, tag="rec", name=f"rec{sb}")
        nc.vector.reciprocal(rec, den)
        nc.vector.tensor_tensor(
            A, A, rec[:, :, :, None].to_broadcast([128, 4, 8, 8]), ALU.mult)

        # AV: o[b, qi, h, d] = sum_ki A[b,h,qi,ki] * vc[b,ki,(h,d)]
        o_a = commp.tile([128, 8, 4, 32], BF16, tag="o_a", name=f"oa{sb}")
        o_b = commp.tile([128, 8, 4, 32], BF16, tag="o_b", name=f"ob{sb}")
        cur = o_a
        for ki in range(8):
            prod2 = commp.tile([128, 8, 4, 32], BF16, tag=f"prodq{ki % 2}",
                              name=f"p2_{sb}_{ki}")
            a_sl = (A[:, :, :, ki]                   # [128, h, qi]
                    .rearrange("p h q -> p q h")     # [128, qi, h]
                    [:, :, :, None].to_broadcast([128, 8, 4, 32]))
            v_bc = vc[ki][:, None, :, :].to_broadcast([128, 8, 4, 32])
            eng = nc.vector if ki % 2 == 0 else nc.gpsimd
            if ki == 0:
                eng.tensor_tensor(cur, v_bc, a_sl, ALU.mult)
            else:
                eng.tensor_tensor(prod2, v_bc, a_sl, ALU.mult)
                nxt = o_b if cur is o_a else o_a
                nc.vector.tensor_tensor(nxt, cur, prod2, ALU.add)
                cur = nxt

        # per block: transpose o, output fc, gated tanh, hx blend
        for j in range(8):
            pst = ps_tr.tile([128, 128], BF16, tag="tr", name=f"otr{j}_{sb}")
            nc.tensor.transpose(pst, cur[:, j], identb)
            otmp = tsmall.tile([128, 128], F32, tag="otmp",
                               name=f"ot{j}_{sb}")
            nc.scalar.copy(otmp, pst)

            psf = ps_small.tile([128, 128], F32, tag="cmm", name=f"psf{j}_{sb}")
            psg = ps_small.tile([128, 128], F32, tag="cmm", name=f"psg{j}_{sb}")
            nc.tensor.matmul(psf, wc["wfc"], otmp, start=True, stop=True)
            nc.tensor.matmul(psg, wc["wgc"], otmp, start=True, stop=True)
            tf = tsmall.tile([128, 128], F32, tag="tf", name=f"tf{j}_{sb}")
            sg = tsmall.tile([128, 128], F32, tag="sg", name=f"sg{j}_{sb}")
            if use_bias:
                bfcol = consts.tile([128, 1], F32, tag="bfcol")
                bgcol = consts.tile([128, 1], F32, tag="bgcol")
                if j == 0 and sb == 0:
                    nc.sync.dma_start(out=bfcol, in_=io["bfc"][:, None])
                    nc.sync.dma_start(out=bgcol, in_=io["bgc"][:, None])
                nc.scalar.activation(tf, psf, ACTF.Tanh, bias=bfcol, scale=1.0)
                nc.scalar.activation(sg, psg, ACTF.Sigmoid, bias=bgcol,
                                     scale=1.0)
            else:
                nc.scalar.activation(tf, psf, ACTF.Tanh)
                nc.scalar.activation(sg, psg, ACTF.Sigmoid)
            # comm = sigmoid(og) * tanh(of), in place on sg
            nc.vector.tensor_tensor(sg, sg, tf, ALU.mult)

            # hx_new = hb + comm; hx_out = hx + mask*(hx_new - hx)
            mexp = ps_small.tile([128, 128], F32, tag="cmm",
                                name=f"mexp{j}_{sb}")
            nc.tensor.matmul(mexp, bigE[:, j, :], miT[:, sbs],
                             start=True, stop=True)
            hb = hbT[j][:, sbs]
            nc.vector.tensor_tensor(hb, hb, sg, ALU.add)
            nc.vector.tensor_tensor(hb, hb, hxT[j][:, sbs], ALU.subtract)
            nc.vector.tensor_tensor(hb, hb, mexp, ALU.mult)
            hout = outp.tile([128, 128], F32, tag="hout", name=f"ho{j}_{sb}")
            nc.vector.tensor_tensor(hout, hb, hxT[j][:, sbs], ALU.add)
            nc.sync.dma_start(out=io["hx_outT"][j * 128:(j + 1) * 128, sbs],
                              in_=hout)


# ---------------------------------------------------------------------------
# Host side
# ---------------------------------------------------------------------------
_BUILD_LOCK = threading.Lock()
_CACHED = {}


def _declare_io(nc):
    io = {}

    def inp(name, shape):
        io[name] = nc.dram_tensor(name, shape, F32, kind="ExternalInput").ap()

    def outp(name, shape):
        io[name] = nc.dram_tensor(name, shape, F32, kind="ExternalOutput").ap()

    inp("xT", (NHID, BPC))
    inp("hxT", (NHID, BPC))
    inp("cxT", (NHID, BPC))
    inp("wq", (BS_OUT, DK_IN))
    inp("wk", (BS_IN, DK_IN))
    inp("wv", (BS_IN, ATT_OUT))
    inp("bv", (ATT_OUT,))
    inp("wihT", (8, 512, 512))
    inp("whhT", (8, 128, 512))
    inp("biasgT", (8, 4, 128))
    inp("bqi", (DK_IN,))
    inp("e16", (16, 16, 128))
    for nm in ("wqc", "wkc", "wvc", "wfc", "wgc"):
        inp(nm, (128, 128))
    for nm in ("bqc", "bkc", "bvc", "bfc", "bgc"):
        inp(nm, (128,))
    outp("hx_outT", (NHID, BPC))
    outp("cx_outT", (NHID, BPC))
    outp("mask_out", (BPC, NHID))
    outp("bm_out", (BPC, 8))
    return io


def _build_nc(use_bias):
    key = bool(use_bias)
    with _BUILD_LOCK:
        if key in _CACHED:
            return _CACHED[key]
        _apply_tile_patch()
        nc = bass.Bass("TRN2")
        io = _declare_io(nc)
        with nc.allow_low_precision("comm attention core in bf16; "
                                     "not on the top-k mask path"):
            with tile.TileContext(nc) as tc:
                with ExitStack() as ctx:
                    build_kernel(ctx, tc, io, use_bias)
        _CACHED[key] = nc
        return nc


def _pack_weights(inputs):
    W_ih = np.asarray(inputs["W_ih"], np.float32)
    W_hh = np.asarray(inputs["W_hh"], np.float32)
    b_ih = np.asarray(inputs["b_ih"], np.float32)
    b_hh = np.asarray(inputs["b_hh"], np.float32)
    wihT = np.empty((8, 512, 512), np.float32)
    whhT = np.empty((8, 128, 512), np.float32)
    biasg = np.empty((8, 4, 128), np.float32)
    for j in range(8):
        for g in range(4):
            rows = slice(g * NHID + j * 128, g * NHID + (j + 1) * 128)
            wihT[j][:, g * 128:(g + 1) * 128] = \
                W_ih[rows, j * 512:(j + 1) * 512].T
            whhT[j][:, g * 128:(g + 1) * 128] = \
                W_hh[rows, j * 128:(j + 1) * 128].T
            biasg[j, g] = (b_ih + b_hh)[rows]
    return wihT, whhT, biasg


def _cf(x):
    return np.ascontiguousarray(np.asarray(x, np.float32))


def _make_e16():
    e = np.zeros((16, 16, 128), np.float32)
    for r in range(16):
        e[r, r, :] = 1.0
    return e


def make_in_maps(inputs):
    inp = _cf(inputs["inp"])
    hx = _cf(inputs["hx"])
    cx = _cf(inputs["cx"])

    wihT, whhT, biasg = _pack_weights(inputs)
    bias_arrs = [inputs[k] for k in
                 ("bq_i", "bk_i", "bv_i", "bq_c", "bk_c", "bv_c",
                  "bf_c", "bg_c", "b_ih", "b_hh")]
    use_bias = bool(any(np.any(np.asarray(a)) for a in bias_arrs))

    shared = {
        "wq": _cf(inputs["Wq_i"]), "wk": _cf(inputs["Wk_i"]),
        "wv": _cf(inputs["Wv_i"]), "bv": _cf(inputs["bv_i"]),
        "wihT": wihT, "whhT": whhT, "biasgT": biasg,
        "bqi": _cf(inputs["bq_i"]),
        "e16": _make_e16(),
        "wqc": _cf(inputs["Wq_c"]), "wkc": _cf(inputs["Wk_c"]),
        "wvc": _cf(inputs["Wv_c"]),
        "bqc": _cf(inputs["bq_c"]), "bkc": _cf(inputs["bk_c"]),
        "bvc": _cf(inputs["bv_c"]),
        "wfc": _cf(inputs["Wf_c"]), "wgc": _cf(inputs["Wg_c"]),
        "bfc": _cf(inputs["bf_c"]), "bgc": _cf(inputs["bg_c"]),
    }

    in_maps = []
    for r in range(NCORES):
        rows = slice(r * BPC, (r + 1) * BPC)
        m = dict(shared)
        m["xT"] = np.ascontiguousarray(inp[rows].T)
        m["hxT"] = np.ascontiguousarray(hx[rows].T)
        m["cxT"] = np.ascontiguousarray(cx[rows].T)
        in_maps.append(m)
    return in_maps, use_bias


def assemble_outputs(results):
    hx_out = np.empty((B, NHID), np.float32)
    cx_out = np.empty((B, NHID), np.float32)
    mask = np.empty((B, NHID), np.float32)
    bm = np.empty((B, 8), np.float32)
    for r in range(NCORES):
        rows = slice(r * BPC, (r + 1) * BPC)
        out = results[r]
        hx_out[rows] = out["hx_outT"].T
        cx_out[rows] = out["cx_outT"].T
        mask[rows] = out["mask_out"]
        bm[rows] = out["bm_out"]
    return hx_out, cx_out, mask, bm.reshape(B, 8, 1)


def kernel(**inputs):
    in_maps, use_bias = make_in_maps(inputs)
    nc = _build_nc(use_bias)
    res = run_bass_kernel_spmd(nc, in_maps, core_ids=list(range(NCORES)))
    return assemble_outputs(res.results)
